# revision 6
# baseline (speedup 1.0000x reference)
"""Trainium2 Bass kernel for KernelAttentionEncoder, v5.

Math per batch element b (N=2048 nodes, D=O=128, H=3 heads, sigma=1,2,4):
  d2[j,i]  = ||c_j - c_i||^2
  E_h      = exp(-d2 / sigma_h^2)        (clip at -20 omitted: ~1e-6 effect)
  A_h      = E_h^T V_h,  S_h[i] = sum_j m_j E_h[j,i]
  out      = (sum_h (A_h/S_h) @ Wo_h) + bo, masked rows zeroed

Data-parallel over B=8 across 8 NeuronCores. Flash-style: NxN matrices
live only as [128, 512] tiles.

v4 keeps every engine near its own roofline by splitting the work:
  - ACT computes only E_0 = exp(-d2) and E_2 = exp(-d2/16) (2 exps/tile).
  - DVE derives E_1 = (E_2^2)^2 via two bf16 squarings (2x DVE mode).
  - The padding mask is folded into the coordinates on the host (masked
    atoms are shifted +1000 in each axis), so E rows of masked j
    underflow to exact 0 against valid i and no mask is needed in the
    denominator; S >= 1 always (diagonal term, or the masked-masked
    block for masked columns). V rows are still zeroed for masked j,
    which kills the masked-masked block in the numerator.
  - Denominator row-sums matmul over PAIR-SUMMED E tiles (halving the
    PE work): pair adds run on the otherwise-idle GpSimd engine (heads
    0/2) and DVE (head 1).
  - The numerator phase consumes E tiles jt-major, matching the
    production order, so PE tracks ACT with ~1 tile of lag.
"""

import numpy as np
from contextlib import ExitStack

import concourse.bass as bass
import concourse.bacc as bacc
import concourse.tile as tile
import concourse.mybir as mybir
from concourse import bass_utils

F32 = mybir.dt.float32
F32R = mybir.dt.float32r
BF16 = mybir.dt.bfloat16

B, N, D, O, H = 8, 2048, 128, 128, 3
SIGMAS = (1.0, 2.0, 4.0)
NJT = N // 128          # 16 j-tiles of 128 (contraction/partition dim)
NIB = 4                 # i-blocks of 512
IB = 512
NSL = IB // 128         # 4 i-slices of 128 per block
NIT = N // 128          # 16 i-tiles total
NPR = NJT // 2          # 8 j-tile pairs for the denominator

_CACHE = {}


def _build_nc(reps=1):
    nc = bacc.Bacc("TRN2", target_bir_lowering=False, debug=False, num_devices=B)

    d_nfT = nc.dram_tensor("nfT", [D, N], F32R, kind="ExternalInput")
    d_cj13 = nc.dram_tensor("cj13", [24, N], BF16, kind="ExternalInput")
    d_ci13 = nc.dram_tensor("ci13", [24, N], BF16, kind="ExternalInput")
    d_wv = nc.dram_tensor("wv", [D, H * O], F32R, kind="ExternalInput")
    d_wo = nc.dram_tensor("wo", [O, H * O], BF16, kind="ExternalInput")
    d_bob4 = nc.dram_tensor("bob4", [128, IB], F32, kind="ExternalInput")
    d_colm = nc.dram_tensor("colm", [128, NJT], F32, kind="ExternalInput")
    d_rowm = nc.dram_tensor("rowm", [128, NIT], F32, kind="ExternalInput")
    d_out = nc.dram_tensor("out", [N, O], F32, kind="ExternalOutput")

    NBLK = NIB * reps

    with tile.TileContext(nc) as tc, ExitStack() as ctx:
        cpool = ctx.enter_context(tc.tile_pool(name="const", bufs=1))
        epool = ctx.enter_context(tc.tile_pool(name="et", bufs=64))
        qpool = ctx.enter_context(tc.tile_pool(name="tsq", bufs=4))
        spool = ctx.enter_context(tc.tile_pool(name="sp", bufs=32))
        mpool = ctx.enter_context(tc.tile_pool(name="mt", bufs=1))
        rpool = ctx.enter_context(tc.tile_pool(name="rs", bufs=4))
        outp = ctx.enter_context(tc.tile_pool(name="outp", bufs=2))
        ps_d2 = ctx.enter_context(tc.tile_pool(name="ps_d2", bufs=4, space="PSUM"))
        ps_acc = ctx.enter_context(tc.tile_pool(name="ps_acc", bufs=2, space="PSUM"))
        ps_s = ctx.enter_context(tc.tile_pool(name="ps_s", bufs=2, space="PSUM"))

        def ctile(nm, shape, dt=F32):
            return cpool.tile(shape, dt, name=nm, tag=nm)

        t_nfT = ctile("t_nfT", [128, N], F32R)
        t_cj13 = [ctile("t_cj13a", [24, N // 2], BF16),
                  ctile("t_cj13b", [24, N // 2], BF16)]
        t_ci13 = [ctile("t_ci13a", [24, N // 2], BF16),
                  ctile("t_ci13b", [24, N // 2], BF16)]
        t_wv = ctile("t_wv", [128, H * O], F32R)
        t_wo = ctile("t_wo", [128, H * O], BF16)
        t_bob4 = ctile("t_bob4", [128, IB])
        t_colm = ctile("t_colm", [128, NJT])
        t_rowm = ctile("t_rowm", [128, NIT])
        t_ones = ctile("t_ones", [128, 128], BF16)
        t_cj0 = ctile("t_cj0", [24, 128], BF16)
        t_ci0 = ctile("t_ci0", [24, IB], BF16)

        # d2 operands first so the first matmuls are not DMA-gated; two
        # queue engines so the loads overlap.
        nc.sync.dma_start(t_cj0[:], d_cj13.ap()[:, 0:128])
        nc.gpsimd.dma_start(t_ci0[:], d_ci13.ap()[:, 0:IB])
        nc.sync.dma_start(t_cj13[0][:], d_cj13.ap()[:, 0:N // 2])
        nc.sync.dma_start(t_cj13[1][:], d_cj13.ap()[:, N // 2:])
        nc.gpsimd.dma_start(t_ci13[0][:], d_ci13.ap()[:, 0:N // 2])
        nc.gpsimd.dma_start(t_ci13[1][:], d_ci13.ap()[:, N // 2:])
        nc.gpsimd.dma_start(t_colm[:], d_colm.ap())
        nc.sync.dma_start(t_nfT[:], d_nfT.ap())
        nc.gpsimd.dma_start(t_wv[:], d_wv.ap())
        nc.gpsimd.dma_start(t_wo[:], d_wo.ap())
        nc.gpsimd.dma_start(t_bob4[:], d_bob4.ap())
        nc.gpsimd.dma_start(t_rowm[:], d_rowm.ap())
        nc.vector.memset(t_ones[:], 1.0)
        # tiny dummy exp: pulls the ACT table load off the critical path
        t_dum = ctile("t_dum", [128, 1], BF16)
        nc.scalar.activation(t_dum[:], t_ones[:, 0:1],
                             mybir.ActivationFunctionType.Exp, scale=-1.0)

        # ---- V phase (emitted interleaved with the first units below):
        # v1[jt][:, h*O:(h+1)*O] = (nfT_jt^T @ Wv_h) * colmask_j
        v1 = []

        def emit_v(jt):
            pv = ps_acc.tile([128, IB], F32, name="pv", tag="acc")
            nc.tensor.matmul(
                pv[:, 0:H * O],
                t_nfT[:, jt * 128:(jt + 1) * 128],
                t_wv[:],
                start=True, stop=True,
            )
            vt = cpool.tile([128, H * O], BF16, name=f"v{jt}", tag=f"v{jt}")
            nc.vector.tensor_scalar(
                vt[:], pv[:, 0:H * O], t_colm[:, jt:jt + 1], None,
                mybir.AluOpType.mult,
            )
            v1.append(vt)

        # ---- per-unit emission: d2 tile -> exp E_0/E_2 (ACT), E_1 via two
        # DVE squarings; on odd jt also the pair-sum tiles for the
        # denominator (heads 0/2 on GpSimd, head 1 on DVE).
        ets = {}
        spairs = {}
        MULT = mybir.AluOpType.mult
        ADD = mybir.AluOpType.add

        def emit_unit(k, jt):
            kb = k % NIB
            i0 = kb * IB
            pd2 = ps_d2.tile([128, IB], F32, name="pd2", tag="d2")
            if k == 0 and jt == 0:
                nc.tensor.matmul(pd2[:], t_cj0[:], t_ci0[:],
                                 start=True, stop=True)
            else:
                cjh, cjo = divmod(jt * 128, N // 2)
                cih, cio = divmod(i0, N // 2)
                nc.tensor.matmul(
                    pd2[:],
                    t_cj13[cjh][:, cjo:cjo + 128],
                    t_ci13[cih][:, cio:cio + IB],
                    start=True, stop=True,
                )
            et2 = epool.tile([128, IB], BF16, name="et2", tag="et")
            nc.scalar.activation(et2[:], pd2[:],
                                 mybir.ActivationFunctionType.Exp,
                                 scale=-1.0 / 16.0)
            et0 = epool.tile([128, IB], BF16, name="et0", tag="et")
            nc.scalar.activation(et0[:], pd2[:],
                                 mybir.ActivationFunctionType.Exp, scale=-1.0)
            et1 = epool.tile([128, IB], BF16, name="et1", tag="et")
            if jt % 2 == 0:
                nc.scalar.activation(et1[:], pd2[:],
                                     mybir.ActivationFunctionType.Exp,
                                     scale=-0.25)
            else:
                tsq = qpool.tile([128, IB], BF16, name="tsq", tag="tsq")
                nc.vector.tensor_tensor(tsq[:], et2[:], et2[:], MULT)
                nc.vector.tensor_tensor(et1[:], tsq[:], tsq[:], MULT)
            ets[(k, 0, jt)] = et0
            ets[(k, 1, jt)] = et1
            ets[(k, 2, jt)] = et2
            if jt % 2 == 1:
                p = jt // 2
                for h, eng in ((0, nc.gpsimd), (1, nc.vector), (2, nc.gpsimd)):
                    sp = spool.tile([128, IB], BF16, name=f"sp{h}", tag="sp")
                    eng.tensor_tensor(sp[:], ets[(k, h, jt - 1)][:],
                                      ets[(k, h, jt)][:], ADD)
                    spairs[(k, h, p)] = sp
                if p % 2 == 1:
                    q = p // 2
                    for h in range(H):
                        sq = spool.tile([128, IB], BF16, name=f"sq{h}",
                                        tag="sq1")
                        nc.vector.tensor_tensor(
                            sq[:], spairs.pop((k, h, 2 * q))[:],
                            spairs.pop((k, h, 2 * q + 1))[:], ADD)
                        spairs[(k, h, "q%d" % q)] = sq

        # ---- main loop over 512-wide i-blocks, software-pipelined one
        # block deep: while block k's numerator accumulates, the pump
        # emits block k+1's d2+exp units AND its denominator row-sum MMs
        # (pair tiles lag two units behind the exps that feed them), and
        # block k-1's output projection slots into the stream. Block
        # boundaries carry no serial S-phase or projection stalls.
        psumS = {}
        proj_pending = []

        QL = ["q0", "q1", "q2", "q3"]
        SPARTS = {0: QL, 1: QL, 2: QL}

        def emit_spart(k, h, i):
            parts = SPARTS[h]
            if i == 0:
                psumS[(k, h)] = ps_s.tile([128, IB], F32, name=f"ps_{h}",
                                          tag="s")
            nc.tensor.matmul(
                psumS[(k, h)][:], t_ones[:], spairs.pop((k, h, parts[i]))[:],
                start=(i == 0), stop=(i == len(parts) - 1),
            )

        def emit_unit_s(k, jt):
            emit_unit(k, jt)

        def finish_s(k):
            for h in range(H):
                for i in range(len(SPARTS[h])):
                    emit_spart(k, h, i)

        def make_proj(k, multiT):
            kb = k % NIB

            def go():
                p3 = ps_s.tile([128, IB], F32, name="p3", tag="s")
                for s in range(NSL):
                    for h in range(H):
                        nc.tensor.matmul(
                            p3[:, s * 128:(s + 1) * 128],
                            multiT[h][:, s * 128:(s + 1) * 128],
                            t_wo[:, h * O:(h + 1) * O],
                            start=(h == 0), stop=(h == H - 1),
                        )
                ab = outp.tile([128, IB], F32, name="ab", tag="ab")
                nc.vector.tensor_tensor(ab[:], p3[:], t_bob4[:], ADD)
                ob = outp.tile([128, IB], F32, name="ob", tag="ob")
                for s in range(NSL):
                    ti = kb * NSL + s
                    nc.vector.tensor_scalar(
                        ob[:, s * 128:(s + 1) * 128],
                        ab[:, s * 128:(s + 1) * 128],
                        t_rowm[:, ti:ti + 1], None, MULT,
                    )
                    eng = nc.sync if s % 2 == 0 else nc.gpsimd
                    eng.dma_start(
                        d_out.ap()[ti * 128:(ti + 1) * 128, :],
                        ob[:, s * 128:(s + 1) * 128],
                    )
            return go

        for k in range(NBLK):
            if k == 0:
                for jt in range(NJT):
                    emit_unit_s(0, jt)
                    emit_v(jt)
                finish_s(0)
            pend = list(range(NJT)) if k + 1 < NBLK else []
            cnt = [0]

            def pump():
                cnt[0] += 1
                if pend and cnt[0] % 3 == 0:
                    emit_unit_s(k + 1, pend.pop(0))

            # reciprocals of this block's (already accumulated) row sums
            rs = []
            for h in range(H):
                r = rpool.tile([128, IB], F32, name=f"rs{h}", tag="rs")
                nc.vector.reciprocal(r[:], psumS.pop((k, h))[:])
                rs.append(r)

            # numerator: two passes (h0,h1 then h2) in steady state, so
            # only two accumulator banks are live; the LAST block runs a
            # single 3-head pass (3rd bank borrowed from the idle S pool)
            # so its tail isn't serialized behind a second pass.
            last = k == NBLK - 1
            p2 = {}
            mtd = {}
            for h in (0, 2):
                p2[h] = ps_acc.tile([128, IB], F32, name=f"p2_{h}", tag="acc")
            if last:
                # single 3-head pass: the 3rd accumulator borrows the idle
                # S-pool bank so the tail isn't serialized behind a 2nd pass
                p2[1] = ps_s.tile([128, IB], F32, name="p2_1", tag="s")
            passA = (0, 2, 1) if last else (0, 2)
            for jt in range(NJT):
                for h in passA:
                    nc.tensor.matmul(
                        p2[h][:], v1[jt][:, h * O:(h + 1) * O],
                        ets[(k, h, jt)][:],
                        start=(jt == 0), stop=(jt == NJT - 1),
                    )
                    pump()
                if jt == 2 and proj_pending:
                    proj_pending.pop(0)()

            for h in passA:
                mt = mpool.tile([128, IB], BF16, name=f"mt{h}", tag=f"mt{h}",
                                bufs=2)
                nc.vector.tensor_tensor(mt[:], p2[h][:], rs[h][:], MULT)
                mtd[h] = mt

            if not last:
                # numerator pass B: head 1 (E tiles come off the DVE chain)
                p2[1] = ps_acc.tile([128, IB], F32, name="p2_1", tag="acc")
                for jt in range(NJT):
                    nc.tensor.matmul(
                        p2[1][:], v1[jt][:, O:2 * O],
                        ets[(k, 1, jt)][:],
                        start=(jt == 0), stop=(jt == NJT - 1),
                    )
                    pump()
                while pend:
                    emit_unit_s(k + 1, pend.pop(0))
                finish_s(k + 1)
                mt = mpool.tile([128, IB], BF16, name="mt1", tag="mt1",
                                bufs=2)
                nc.vector.tensor_tensor(mt[:], p2[1][:], rs[1][:], MULT)
                mtd[1] = mt
            multiT = [mtd[0], mtd[1], mtd[2]]
            for h in range(H):
                for jt in range(NJT):
                    del ets[(k, h, jt)]

            proj_pending.append(make_proj(k, multiT))
            if k == NBLK - 1:
                while proj_pending:
                    proj_pending.pop(0)()

    nc.compile()
    return nc


def _prepare_core_inputs(nf_b, c_b, mask_b, Wv, Wo, bo):
    import ml_dtypes

    bf16 = ml_dtypes.bfloat16

    def split3(x):
        """x (fp32) -> 3 bf16 parts summing to x within ~2^-27 relative."""
        h = x.astype(bf16)
        r1 = x - h.astype(np.float32)
        m = r1.astype(bf16)
        l = (r1 - m.astype(np.float32)).astype(bf16)
        return h, m, l

    mask = np.asarray(mask_b)
    c = c_b.astype(np.float32).copy()               # [N, 3]
    # Fold the padding mask into the coordinates: masked atoms go far
    # away, so their E rows underflow to exact 0 against valid atoms.
    c[mask] += 1000.0
    c2 = (c * c).sum(axis=1, dtype=np.float32)      # [N]
    ch, cm, cl = split3(c)                          # [N, 3] each
    c2h, c2m, c2l = split3(c2)                      # [N] each
    one = np.ones((1, N), bf16)
    hT, mT, lT = ch.T, cm.T, cl.T                   # [3, N]

    def neg2(x):
        return (-2.0 * x.astype(np.float32)).astype(bf16)  # exact scaling

    # d2[j,i] = |cj|^2 + |ci|^2 - 2 cj.ci with cj.ci expanded over the
    # split pairs (h,h),(h,m),(m,h),(h,l),(l,h),(m,m); dropped terms are
    # O(2^-27). 18 cross rows + 3 |cj|^2 rows + 3 |ci|^2 rows = 24.
    cj13 = np.concatenate(
        [hT, hT, mT, hT, lT, mT,
         c2h[None], c2m[None], c2l[None], one, one, one]
    ).astype(bf16)
    ci13 = np.concatenate(
        [neg2(hT), neg2(mT), neg2(hT), neg2(lT), neg2(hT), neg2(mT),
         one, one, one, c2h[None], c2m[None], c2l[None]]
    ).astype(bf16)
    valid = (~mask).astype(np.float32)
    vT = np.ascontiguousarray(valid.reshape(NJT, 128).T)  # [128, 16]
    Wv32 = np.asarray(Wv, dtype=np.float32)               # [H, D, O]
    Wo32 = np.asarray(Wo, dtype=np.float32).reshape(H, O, O)
    return {
        "nfT": np.ascontiguousarray(nf_b.astype(np.float32).T),
        "cj13": np.ascontiguousarray(cj13),
        "ci13": np.ascontiguousarray(ci13),
        "wv": np.ascontiguousarray(
            Wv32.transpose(1, 0, 2).reshape(D, H * O)
        ),
        "wo": np.ascontiguousarray(
            Wo32.transpose(1, 0, 2).reshape(O, H * O).astype(bf16)
        ),
        "bob4": np.ascontiguousarray(
            np.tile(bo.astype(np.float32)[None, :], (128, NSL))
        ),
        "colm": vT,
        "rowm": vT.copy(),
    }


def kernel(node_features, coordinates, masked_elements, Wv, Wo, bo):
    node_features = np.asarray(node_features)
    coordinates = np.asarray(coordinates)
    masked_elements = np.asarray(masked_elements)
    Wv, Wo, bo = np.asarray(Wv), np.asarray(Wo), np.asarray(bo)

    if "nc" not in _CACHE:
        _CACHE["nc"] = _build_nc()
    nc = _CACHE["nc"]

    in_maps = [
        _prepare_core_inputs(
            node_features[b], coordinates[b], masked_elements[b], Wv, Wo, bo
        )
        for b in range(B)
    ]
    res = bass_utils.run_bass_kernel_spmd(nc, in_maps, core_ids=list(range(B)))
    out = np.stack([res.results[b]["out"] for b in range(B)])
    return out.astype(np.float32)


# revision 7
# speedup vs baseline: 1.0164x; 1.0164x over previous
"""Trainium2 Bass kernel for KernelAttentionEncoder, v5.

Math per batch element b (N=2048 nodes, D=O=128, H=3 heads, sigma=1,2,4):
  d2[j,i]  = ||c_j - c_i||^2
  E_h      = exp(-d2 / sigma_h^2)        (clip at -20 omitted: ~1e-6 effect)
  A_h      = E_h^T V_h,  S_h[i] = sum_j m_j E_h[j,i]
  out      = (sum_h (A_h/S_h) @ Wo_h) + bo, masked rows zeroed

Data-parallel over B=8 across 8 NeuronCores. Flash-style: NxN matrices
live only as [128, 512] tiles.

v4 keeps every engine near its own roofline by splitting the work:
  - ACT computes only E_0 = exp(-d2) and E_2 = exp(-d2/16) (2 exps/tile).
  - DVE derives E_1 = (E_2^2)^2 via two bf16 squarings (2x DVE mode).
  - The padding mask is folded into the coordinates on the host (masked
    atoms are shifted +1000 in each axis), so E rows of masked j
    underflow to exact 0 against valid i and no mask is needed in the
    denominator; S >= 1 always (diagonal term, or the masked-masked
    block for masked columns). V rows are still zeroed for masked j,
    which kills the masked-masked block in the numerator.
  - Denominator row-sums matmul over PAIR-SUMMED E tiles (halving the
    PE work): pair adds run on the otherwise-idle GpSimd engine (heads
    0/2) and DVE (head 1).
  - The numerator phase consumes E tiles jt-major, matching the
    production order, so PE tracks ACT with ~1 tile of lag.
"""

import numpy as np
from contextlib import ExitStack

import concourse.bass as bass
import concourse.bacc as bacc
import concourse.tile as tile
import concourse.mybir as mybir
from concourse import bass_utils

F32 = mybir.dt.float32
F32R = mybir.dt.float32r
BF16 = mybir.dt.bfloat16

B, N, D, O, H = 8, 2048, 128, 128, 3
SIGMAS = (1.0, 2.0, 4.0)
NJT = N // 128          # 16 j-tiles of 128 (contraction/partition dim)
NIB = 4                 # i-blocks of 512
IB = 512
NSL = IB // 128         # 4 i-slices of 128 per block
NIT = N // 128          # 16 i-tiles total
NPR = NJT // 2          # 8 j-tile pairs for the denominator

_CACHE = {}


def _build_nc(reps=1):
    nc = bacc.Bacc("TRN2", target_bir_lowering=False, debug=False, num_devices=B)

    d_nfT = nc.dram_tensor("nfT", [D, N], F32R, kind="ExternalInput")
    d_cj13 = nc.dram_tensor("cj13", [24, N], BF16, kind="ExternalInput")
    d_ci13 = nc.dram_tensor("ci13", [24, N], BF16, kind="ExternalInput")
    d_wv = nc.dram_tensor("wv", [D, H * O], F32R, kind="ExternalInput")
    d_wo = nc.dram_tensor("wo", [O, H * O], BF16, kind="ExternalInput")
    d_bob4 = nc.dram_tensor("bob4", [128, IB], F32, kind="ExternalInput")
    d_colm = nc.dram_tensor("colm", [128, NJT], F32, kind="ExternalInput")
    d_rowm = nc.dram_tensor("rowm", [128, NIT], F32, kind="ExternalInput")
    d_out = nc.dram_tensor("out", [N, O], F32, kind="ExternalOutput")

    NBLK = NIB * reps

    with tile.TileContext(nc) as tc, ExitStack() as ctx:
        cpool = ctx.enter_context(tc.tile_pool(name="const", bufs=1))
        epool = ctx.enter_context(tc.tile_pool(name="et", bufs=64))
        qpool = ctx.enter_context(tc.tile_pool(name="tsq", bufs=4))
        spool = ctx.enter_context(tc.tile_pool(name="sp", bufs=32))
        mpool = ctx.enter_context(tc.tile_pool(name="mt", bufs=1))
        rpool = ctx.enter_context(tc.tile_pool(name="rs", bufs=4))
        outp = ctx.enter_context(tc.tile_pool(name="outp", bufs=2))
        ps_d2 = ctx.enter_context(tc.tile_pool(name="ps_d2", bufs=4, space="PSUM"))
        ps_acc = ctx.enter_context(tc.tile_pool(name="ps_acc", bufs=2, space="PSUM"))
        ps_s = ctx.enter_context(tc.tile_pool(name="ps_s", bufs=2, space="PSUM"))

        def ctile(nm, shape, dt=F32):
            return cpool.tile(shape, dt, name=nm, tag=nm)

        t_nfT = ctile("t_nfT", [128, N], F32R)
        t_cj13 = [ctile("t_cj13a", [24, N // 2], BF16),
                  ctile("t_cj13b", [24, N // 2], BF16)]
        t_ci13 = [ctile("t_ci13a", [24, N // 2], BF16),
                  ctile("t_ci13b", [24, N // 2], BF16)]
        t_wv = ctile("t_wv", [128, H * O], F32R)
        t_wo = ctile("t_wo", [128, H * O], BF16)
        t_bob4 = ctile("t_bob4", [128, IB])
        t_colm = ctile("t_colm", [128, NJT])
        t_rowm = ctile("t_rowm", [128, NIT])
        t_ones = ctile("t_ones", [128, 128], BF16)
        t_cj0 = ctile("t_cj0", [24, 128], BF16)
        t_ci0 = ctile("t_ci0", [24, IB], BF16)

        # d2 operands first so the first matmuls are not DMA-gated; two
        # queue engines so the loads overlap.
        nc.sync.dma_start(t_cj0[:], d_cj13.ap()[:, 0:128])
        nc.gpsimd.dma_start(t_ci0[:], d_ci13.ap()[:, 0:IB])
        nc.sync.dma_start(t_cj13[0][:], d_cj13.ap()[:, 0:N // 2])
        nc.sync.dma_start(t_cj13[1][:], d_cj13.ap()[:, N // 2:])
        nc.gpsimd.dma_start(t_ci13[0][:], d_ci13.ap()[:, 0:N // 2])
        nc.gpsimd.dma_start(t_ci13[1][:], d_ci13.ap()[:, N // 2:])
        nc.gpsimd.dma_start(t_colm[:], d_colm.ap())
        nc.sync.dma_start(t_nfT[:], d_nfT.ap())
        nc.gpsimd.dma_start(t_wv[:], d_wv.ap())
        nc.gpsimd.dma_start(t_wo[:], d_wo.ap())
        nc.gpsimd.dma_start(t_bob4[:], d_bob4.ap())
        nc.gpsimd.dma_start(t_rowm[:], d_rowm.ap())
        nc.vector.memset(t_ones[:], 1.0)
        # tiny dummy exp: pulls the ACT table load off the critical path
        t_dum = ctile("t_dum", [128, 1], BF16)
        nc.scalar.activation(t_dum[:], t_ones[:, 0:1],
                             mybir.ActivationFunctionType.Exp, scale=-1.0)

        # ---- V phase (emitted interleaved with the first units below):
        # v1[jt][:, h*O:(h+1)*O] = (nfT_jt^T @ Wv_h) * colmask_j
        v1 = []

        def emit_v(jt):
            pv = ps_acc.tile([128, IB], F32, name="pv", tag="acc")
            nc.tensor.matmul(
                pv[:, 0:H * O],
                t_nfT[:, jt * 128:(jt + 1) * 128],
                t_wv[:],
                start=True, stop=True,
            )
            vt = cpool.tile([128, H * O], BF16, name=f"v{jt}", tag=f"v{jt}")
            nc.vector.tensor_scalar(
                vt[:], pv[:, 0:H * O], t_colm[:, jt:jt + 1], None,
                mybir.AluOpType.mult,
            )
            v1.append(vt)

        # ---- per-unit emission: d2 tile -> exp E_0/E_2 (ACT), E_1 via two
        # DVE squarings; on odd jt also the pair-sum tiles for the
        # denominator (heads 0/2 on GpSimd, head 1 on DVE).
        ets = {}
        spairs = {}
        MULT = mybir.AluOpType.mult
        ADD = mybir.AluOpType.add

        def emit_unit(k, jt):
            kb = k % NIB
            i0 = kb * IB
            pd2 = ps_d2.tile([128, IB], F32, name="pd2", tag="d2")
            if k == 0 and jt == 0:
                nc.tensor.matmul(pd2[:], t_cj0[:], t_ci0[:],
                                 start=True, stop=True)
            else:
                cjh, cjo = divmod(jt * 128, N // 2)
                cih, cio = divmod(i0, N // 2)
                nc.tensor.matmul(
                    pd2[:],
                    t_cj13[cjh][:, cjo:cjo + 128],
                    t_ci13[cih][:, cio:cio + IB],
                    start=True, stop=True,
                )
            et2 = epool.tile([128, IB], BF16, name="et2", tag="et")
            nc.scalar.activation(et2[:], pd2[:],
                                 mybir.ActivationFunctionType.Exp,
                                 scale=-1.0 / 16.0)
            et0 = epool.tile([128, IB], BF16, name="et0", tag="et")
            nc.scalar.activation(et0[:], pd2[:],
                                 mybir.ActivationFunctionType.Exp, scale=-1.0)
            et1 = epool.tile([128, IB], BF16, name="et1", tag="et")
            if jt % 2 == 0 and jt != 14:
                nc.scalar.activation(et1[:], pd2[:],
                                     mybir.ActivationFunctionType.Exp,
                                     scale=-0.25)
            else:
                tsq = qpool.tile([128, IB], BF16, name="tsq", tag="tsq")
                nc.vector.tensor_tensor(tsq[:], et2[:], et2[:], MULT)
                nc.vector.tensor_tensor(et1[:], tsq[:], tsq[:], MULT)
            ets[(k, 0, jt)] = et0
            ets[(k, 1, jt)] = et1
            ets[(k, 2, jt)] = et2
            if jt % 2 == 1:
                p = jt // 2
                for h, eng in ((0, nc.gpsimd), (1, nc.vector), (2, nc.gpsimd)):
                    sp = spool.tile([128, IB], BF16, name=f"sp{h}", tag="sp")
                    eng.tensor_tensor(sp[:], ets[(k, h, jt - 1)][:],
                                      ets[(k, h, jt)][:], ADD)
                    spairs[(k, h, p)] = sp
                if p % 2 == 1:
                    q = p // 2
                    for h in range(H):
                        sq = spool.tile([128, IB], BF16, name=f"sq{h}",
                                        tag="sq1")
                        nc.vector.tensor_tensor(
                            sq[:], spairs.pop((k, h, 2 * q))[:],
                            spairs.pop((k, h, 2 * q + 1))[:], ADD)
                        spairs[(k, h, "q%d" % q)] = sq

        # ---- main loop over 512-wide i-blocks, software-pipelined one
        # block deep: while block k's numerator accumulates, the pump
        # emits block k+1's d2+exp units AND its denominator row-sum MMs
        # (pair tiles lag two units behind the exps that feed them), and
        # block k-1's output projection slots into the stream. Block
        # boundaries carry no serial S-phase or projection stalls.
        psumS = {}
        proj_pending = []

        QL = ["q0", "q1", "q2", "q3"]
        SPARTS = {0: QL, 1: QL, 2: QL}

        def emit_spart(k, h, i):
            parts = SPARTS[h]
            if i == 0:
                psumS[(k, h)] = ps_s.tile([128, IB], F32, name=f"ps_{h}",
                                          tag="s")
            nc.tensor.matmul(
                psumS[(k, h)][:], t_ones[:], spairs.pop((k, h, parts[i]))[:],
                start=(i == 0), stop=(i == len(parts) - 1),
            )

        def emit_unit_s(k, jt):
            emit_unit(k, jt)

        def finish_s(k):
            for h in range(H):
                for i in range(len(SPARTS[h])):
                    emit_spart(k, h, i)

        def make_proj(k, multiT):
            kb = k % NIB

            def go():
                p3 = ps_s.tile([128, IB], F32, name="p3", tag="s")
                for s in range(NSL):
                    for h in range(H):
                        nc.tensor.matmul(
                            p3[:, s * 128:(s + 1) * 128],
                            multiT[h][:, s * 128:(s + 1) * 128],
                            t_wo[:, h * O:(h + 1) * O],
                            start=(h == 0), stop=(h == H - 1),
                        )
                ab = outp.tile([128, IB], F32, name="ab", tag="ab")
                nc.vector.tensor_tensor(ab[:], p3[:], t_bob4[:], ADD)
                ob = outp.tile([128, IB], F32, name="ob", tag="ob")
                for s in range(NSL):
                    ti = kb * NSL + s
                    nc.vector.tensor_scalar(
                        ob[:, s * 128:(s + 1) * 128],
                        ab[:, s * 128:(s + 1) * 128],
                        t_rowm[:, ti:ti + 1], None, MULT,
                    )
                    nc.sync.dma_start(
                        d_out.ap()[ti * 128:(ti + 1) * 128, :],
                        ob[:, s * 128:(s + 1) * 128],
                    )
            return go

        for k in range(NBLK):
            if k == 0:
                for jt in range(NJT):
                    emit_unit_s(0, jt)
                    emit_v(jt)
                finish_s(0)
            pend = list(range(NJT)) if k + 1 < NBLK else []
            cnt = [0]

            def pump():
                cnt[0] += 1
                if pend and cnt[0] % 3 == 0:
                    emit_unit_s(k + 1, pend.pop(0))

            # reciprocals of this block's (already accumulated) row sums
            rs = []
            for h in range(H):
                r = rpool.tile([128, IB], F32, name=f"rs{h}", tag="rs")
                nc.vector.reciprocal(r[:], psumS.pop((k, h))[:])
                rs.append(r)

            # numerator: two passes (h0,h1 then h2) in steady state, so
            # only two accumulator banks are live; the LAST block runs a
            # single 3-head pass (3rd bank borrowed from the idle S pool)
            # so its tail isn't serialized behind a second pass.
            last = k == NBLK - 1
            p2 = {}
            mtd = {}
            for h in (0, 2):
                p2[h] = ps_acc.tile([128, IB], F32, name=f"p2_{h}", tag="acc")
            if last:
                # single 3-head pass: the 3rd accumulator borrows the idle
                # S-pool bank so the tail isn't serialized behind a 2nd pass
                p2[1] = ps_s.tile([128, IB], F32, name="p2_1", tag="s")
            passA = (0, 2, 1) if last else (0, 2)
            for jt in range(NJT):
                for h in passA:
                    nc.tensor.matmul(
                        p2[h][:], v1[jt][:, h * O:(h + 1) * O],
                        ets[(k, h, jt)][:],
                        start=(jt == 0), stop=(jt == NJT - 1),
                    )
                    pump()
                if jt == 2 and proj_pending:
                    proj_pending.pop(0)()

            for h in passA:
                mt = mpool.tile([128, IB], BF16, name=f"mt{h}", tag=f"mt{h}",
                                bufs=2)
                nc.vector.tensor_tensor(mt[:], p2[h][:], rs[h][:], MULT)
                mtd[h] = mt

            if not last:
                # numerator pass B: head 1 (E tiles come off the DVE chain)
                p2[1] = ps_acc.tile([128, IB], F32, name="p2_1", tag="acc")
                for jt in range(NJT):
                    nc.tensor.matmul(
                        p2[1][:], v1[jt][:, O:2 * O],
                        ets[(k, 1, jt)][:],
                        start=(jt == 0), stop=(jt == NJT - 1),
                    )
                    pump()
                while pend:
                    emit_unit_s(k + 1, pend.pop(0))
                finish_s(k + 1)
                mt = mpool.tile([128, IB], BF16, name="mt1", tag="mt1",
                                bufs=2)
                nc.vector.tensor_tensor(mt[:], p2[1][:], rs[1][:], MULT)
                mtd[1] = mt
            multiT = [mtd[0], mtd[1], mtd[2]]
            for h in range(H):
                for jt in range(NJT):
                    del ets[(k, h, jt)]

            proj_pending.append(make_proj(k, multiT))
            if k == NBLK - 1:
                while proj_pending:
                    proj_pending.pop(0)()

    nc.compile()
    return nc


def _prepare_core_inputs(nf_b, c_b, mask_b, Wv, Wo, bo):
    import ml_dtypes

    bf16 = ml_dtypes.bfloat16

    def split3(x):
        """x (fp32) -> 3 bf16 parts summing to x within ~2^-27 relative."""
        h = x.astype(bf16)
        r1 = x - h.astype(np.float32)
        m = r1.astype(bf16)
        l = (r1 - m.astype(np.float32)).astype(bf16)
        return h, m, l

    mask = np.asarray(mask_b)
    c = c_b.astype(np.float32).copy()               # [N, 3]
    # Fold the padding mask into the coordinates: masked atoms go far
    # away, so their E rows underflow to exact 0 against valid atoms.
    c[mask] += 1000.0
    c2 = (c * c).sum(axis=1, dtype=np.float32)      # [N]
    ch, cm, cl = split3(c)                          # [N, 3] each
    c2h, c2m, c2l = split3(c2)                      # [N] each
    one = np.ones((1, N), bf16)
    hT, mT, lT = ch.T, cm.T, cl.T                   # [3, N]

    def neg2(x):
        return (-2.0 * x.astype(np.float32)).astype(bf16)  # exact scaling

    # d2[j,i] = |cj|^2 + |ci|^2 - 2 cj.ci with cj.ci expanded over the
    # split pairs (h,h),(h,m),(m,h),(h,l),(l,h),(m,m); dropped terms are
    # O(2^-27). 18 cross rows + 3 |cj|^2 rows + 3 |ci|^2 rows = 24.
    cj13 = np.concatenate(
        [hT, hT, mT, hT, lT, mT,
         c2h[None], c2m[None], c2l[None], one, one, one]
    ).astype(bf16)
    ci13 = np.concatenate(
        [neg2(hT), neg2(mT), neg2(hT), neg2(lT), neg2(hT), neg2(mT),
         one, one, one, c2h[None], c2m[None], c2l[None]]
    ).astype(bf16)
    valid = (~mask).astype(np.float32)
    vT = np.ascontiguousarray(valid.reshape(NJT, 128).T)  # [128, 16]
    Wv32 = np.asarray(Wv, dtype=np.float32)               # [H, D, O]
    Wo32 = np.asarray(Wo, dtype=np.float32).reshape(H, O, O)
    return {
        "nfT": np.ascontiguousarray(nf_b.astype(np.float32).T),
        "cj13": np.ascontiguousarray(cj13),
        "ci13": np.ascontiguousarray(ci13),
        "wv": np.ascontiguousarray(
            Wv32.transpose(1, 0, 2).reshape(D, H * O)
        ),
        "wo": np.ascontiguousarray(
            Wo32.transpose(1, 0, 2).reshape(O, H * O).astype(bf16)
        ),
        "bob4": np.ascontiguousarray(
            np.tile(bo.astype(np.float32)[None, :], (128, NSL))
        ),
        "colm": vT,
        "rowm": vT.copy(),
    }


def kernel(node_features, coordinates, masked_elements, Wv, Wo, bo):
    node_features = np.asarray(node_features)
    coordinates = np.asarray(coordinates)
    masked_elements = np.asarray(masked_elements)
    Wv, Wo, bo = np.asarray(Wv), np.asarray(Wo), np.asarray(bo)

    if "nc" not in _CACHE:
        _CACHE["nc"] = _build_nc()
    nc = _CACHE["nc"]

    in_maps = [
        _prepare_core_inputs(
            node_features[b], coordinates[b], masked_elements[b], Wv, Wo, bo
        )
        for b in range(B)
    ]
    res = bass_utils.run_bass_kernel_spmd(nc, in_maps, core_ids=list(range(B)))
    out = np.stack([res.results[b]["out"] for b in range(B)])
    return out.astype(np.float32)
